# revision 19
# baseline (speedup 1.0000x reference)
"""LookAheadMask kernel for Trainium2.

out[b, r, c] = 1.0 if c > r else x[b, r, c], for x of shape (8, 4096, 4096) f32.

Sharding: batch dim across 8 NeuronCores (data parallel, no communication).

Per-core plan (matrix is S x S, S=4096, row-blocks of P=128), raw bass.

Trace facts this schedule is built on (ntff profiles of prior versions):
  - 16 SDMA engines serve both HWDGE rings, round-robin per packet; a
    queue's byte share is proportional to its packet size vs the other
    queue's, so small-packet streams starve next to big ones.
  - SBUF->DRAM big packets run ~26.5 B/ns/engine; D2D copies ~20 B/ns.
  - A trailing 4096x512B scatter is descriptor-generation limited
    (~10.5 ns/desc = 43 us) with idle engines - this version has none.
  - Engine 15 runs ~19% slower on SBUF-sourced streams; any barrier
    exposes it as a stall, so the schedule has NO mid-kernel barriers.

Structure:
  - strictly-lower region: 31 D2D copies (ascending size) on the SP ring,
    overlapped with the diag gather (1 KB packets) on the ACT ring.
  - diagonal 128x128 blocks ride the upper writes: UB[128, 32*512] holds
    32 slots of 512 cols, all ones except cols [0:128) of slot i = the
    affine-selected diag block i (one gpsimd affine_select, 3D APs).
    Merged write i covers out[r0:r0+P, r0:r0+512] from slot i (2 KB
    descriptors); plain ones DMAs cover out[r0:r0+P, r0+512:S] from a
    ones tile, ungated by the gather/affine.

Engine programs:
  SP    : 31 copies asc | wait affine | 16 even merged writes | drain
  ACT   : gather | wait memset | 28 plain ones asc | wait affine |
          16 odd merged writes | drain
  GPSIMD: wait gather+memset; one affine_select for all 32 slots
  VECTOR: memset UB then ones tile (~17 us, hidden under copies)

HBM traffic/core: ~35 MiB read + 64 MiB write, ~71 MB through the SDMA
engines, no descriptor-generation tail, one straggler exposure at the end.
"""

import numpy as np

from concourse import bass, mybir
from concourse.bass_utils import run_bass_kernel_spmd

S = 4096
P = 128
NB = S // P  # 32
N_CORES = 8
WU = 1024  # merged diag+ones write width (4 KB descriptors)
PO = S - WU  # 3072: plain-ones tile width
NPLAIN = (S - WU) // P  # 24: blocks with cols past r0+WU

DSEM_TARGET = 16 * (31 + NPLAIN)  # copies + plain-ones
USEM_TARGET = 16 * NB  # merged writes

_cached_nc = None


def _build():
    global _cached_nc
    if _cached_nc is not None:
        return _cached_nc

    nc = bass.Bass()
    x = nc.dram_tensor("x", [S, S], mybir.dt.float32, kind="ExternalInput")
    out = nc.dram_tensor("out", [S, S], mybir.dt.float32, kind="ExternalOutput")

    with (
        nc.Block() as block,
        nc.semaphore("dsem") as dsem,  # copy + plain-ones DMA completions
        nc.semaphore("gsem") as gsem,  # diag gather done
        nc.semaphore("msem") as msem,  # memsets done
        nc.semaphore("asem") as asem,  # affine_select done
        nc.semaphore("usem") as usem,  # merged-write DMA completions
        nc.sbuf_tensor("ub", [P, NB * WU], mybir.dt.float32) as ub,
        nc.sbuf_tensor("ones", [P, PO], mybir.dt.float32) as ones,
        nc.sbuf_tensor("diag_in", [P, NB * P], mybir.dt.float32) as diag_in,
    ):

        def merged_write(eng, i):
            """Block-row i's diag block + first ones cols in one DMA."""
            r0 = i * P
            w = min(WU, S - r0)
            eng.dma_start(
                out=out[r0 : r0 + P, r0 : r0 + w],
                in_=ub[:, i * WU : i * WU + w],
            ).then_inc(usem, 16)

        @block.vector
        def _(vector: bass.BassVectorEngine):
            # ones first: it ungates the plain-ones stream at ~5 us so
            # SBUF writes mix with the D2D copies early (raises D2D rate).
            vector.memset(ones[:, :], 1.0).then_inc(msem, 1)
            vector.memset(ub[:, :], 1.0).then_inc(msem, 1)

        def plain_ones(eng, i):
            r0 = i * P
            eng.dma_start(
                out=out[r0 : r0 + P, r0 + WU : S],
                in_=ones[:, : S - r0 - WU],
            ).then_inc(dsem, 16)

        @block.sync
        def _(sync: bass.BassEngine):
            for i in range(1, NB):  # ascending size D2D copies
                r0 = i * P
                sync.dma_start(
                    out=out[r0 : r0 + P, 0:r0], in_=x[r0 : r0 + P, 0:r0]
                ).then_inc(dsem, 16)
            sync.wait_ge(asem, 1)
            for i in range(0, 4):  # ring balance: SP 34.6 MB, ACT 34.7 MB
                merged_write(sync, i)
            sync.wait_ge(dsem, DSEM_TARGET)
            sync.wait_ge(usem, USEM_TARGET)

        @block.scalar
        def _(scalar: bass.BassEngine):
            # Exact-window diag gather: one DMA, 512 B descriptors, all 32
            # blocks (block b at element offset b*(P*S+P)).
            scalar.dma_start(
                out=diag_in[:, :],
                in_=bass.AP(x, 0, [[S, P], [P * S + P, NB], [1, P]]),
            ).then_inc(gsem, 16)
            scalar.wait_ge(msem, 1)
            for i in range(0, NPLAIN):
                plain_ones(scalar, i)
            scalar.wait_ge(asem, 1)
            for i in range(4, NB):
                merged_write(scalar, i)
            scalar.wait_ge(dsem, DSEM_TARGET)
            scalar.wait_ge(usem, USEM_TARGET)

        @block.gpsimd
        def _(gpsimd: bass.BassGpSimd):
            gpsimd.wait_ge(gsem, 16)
            gpsimd.wait_ge(msem, 2)
            # iota[p, (b, c)] = p - c; keep x where >= 0 (at/below diag),
            # else 1.0. Output lands at the head of UB slot b.
            gpsimd.affine_select(
                out=bass.AP(ub, 0, [[NB * WU, P], [WU, NB], [1, P]]),
                in_=bass.AP(diag_in, 0, [[NB * P, P], [P, NB], [1, P]]),
                pattern=[[0, NB], [-1, P]],
                base=0,
                channel_multiplier=1,
                compare_op=mybir.AluOpType.is_ge,
                fill=1.0,
            ).then_inc(asem, 1)

    _cached_nc = nc
    return nc


def _run(x_full: np.ndarray, trace: bool = False):
    nc = _build()
    x_full = np.asarray(x_full, dtype=np.float32)
    in_maps = [{"x": x_full[i]} for i in range(N_CORES)]
    res = run_bass_kernel_spmd(nc, in_maps, list(range(N_CORES)), trace=trace)
    out = np.stack([res.results[i]["out"] for i in range(N_CORES)], axis=0)
    return out, res


def kernel(x: np.ndarray) -> np.ndarray:
    out, _ = _run(x, trace=False)
    return out
